# revision 9
# baseline (speedup 1.0000x reference)
"""Trainium2 Bass kernel for a top-1-routed MoE layer (B=2, S=2048, H=2048, E=3, F=8192).

Strategy (8 NeuronCores):
  Launch A: router in fp32, data-parallel over tokens (512/core).
  Host:     argmax + counting-sort of tokens by expert; pick a uniform
            per-core token quota u and give each core one expert segment
            (single SPMD program; per-core variation is input data only).
  Launch B: expert FFN in bf16 with fp32 accumulation, gelu on ScalarE LUT,
            fused bias, fp32 residual add. Host scatters the disjoint
            token shards back into the full output (pure unshard).
"""

import sys

if "/opt/trn_rl_repo" not in sys.path:
    sys.path.insert(0, "/opt/trn_rl_repo")

import numpy as np
import ml_dtypes

import concourse.bass as bass  # noqa: F401  (bass types used via bacc/tile)
import concourse.tile as tile
from concourse import bacc, mybir
from concourse.bass_utils import run_bass_kernel_spmd

NC = 8
B, S, H, E = 2, 2048, 2048, 3
F = 4 * H
N = B * S                      # 4096 tokens
RTOK = N // NC                 # router tokens per core
HK = H // 128                  # 16 k-tiles over H
FM = F // 128                  # 64 m-tiles over F
RM = (H // 2) // 128           # 8 m-tiles over router hidden (1024)

F32 = mybir.dt.float32
BF16 = mybir.dt.bfloat16
BF = ml_dtypes.bfloat16

_cache = {}


# ----------------------------------------------------------------- router ---
def _build_router(repeat=1):
    key = ("router", repeat)
    if key in _cache:
        return _cache[key]
    nc = bacc.Bacc("TRN2", target_bir_lowering=False, debug=False,
                   enable_asserts=False, num_devices=NC)
    xt = nc.dram_tensor("xt", [H, RTOK], F32, kind="ExternalInput").ap()
    wr1t = nc.dram_tensor("wr1t", [RM, 128, HK, 128], F32, kind="ExternalInput").ap()
    br1t = nc.dram_tensor("br1t", [128, RM], F32, kind="ExternalInput").ap()
    wr2t = nc.dram_tensor("wr2t", [128, RM, E], F32, kind="ExternalInput").ap()
    br2t = nc.dram_tensor("br2t", [E, 1], F32, kind="ExternalInput").ap()
    logits = nc.dram_tensor("logits", [E, RTOK], F32, kind="ExternalOutput").ap()

    with tile.TileContext(nc) as tc:
        with tc.tile_pool(name="xp", bufs=1) as xp, \
             tc.tile_pool(name="wp", bufs=3) as wp, \
             tc.tile_pool(name="cp", bufs=1) as cp, \
             tc.tile_pool(name="r1p", bufs=1) as r1p, \
             tc.tile_pool(name="op", bufs=1) as op, \
             tc.tile_pool(name="ps", bufs=2, space="PSUM") as psp, \
             tc.tile_pool(name="ps2", bufs=1, space="PSUM") as psp2:
            xtile = xp.tile([128, HK, RTOK], F32)
            nc.sync.dma_start(out=xtile[:],
                              in_=xt.rearrange("(kt p) n -> p kt n", p=128))
            b1s = cp.tile([128, RM], F32)
            nc.sync.dma_start(out=b1s[:], in_=br1t[:, :])
            w2s = cp.tile([128, RM, E], F32)
            nc.sync.dma_start(out=w2s[:], in_=wr2t[:, :, :])
            b2s = cp.tile([E, 1], F32)
            nc.sync.dma_start(out=b2s[:], in_=br2t[:, :])

            for _rep in range(repeat):
                r1 = r1p.tile([128, RM, RTOK], F32, tag="r1", name="r1")
                for m in range(RM):
                    wslab = wp.tile([128, HK, 128], F32, tag="wslab", name="wslab")
                    nc.sync.dma_start(out=wslab[:], in_=wr1t[m])
                    ps = psp.tile([128, RTOK], F32, tag="ps", name="ps")
                    for k in range(HK):
                        nc.tensor.matmul(ps[:], wslab[:, k, :], xtile[:, k, :],
                                         start=(k == 0), stop=(k == HK - 1))
                    nc.scalar.activation(r1[:, m, :], ps[:],
                                         mybir.ActivationFunctionType.Relu,
                                         bias=b1s[:, m:m+1])
                ps2 = psp2.tile([E, RTOK], F32, tag="ps2", name="ps2")
                for k in range(RM):
                    nc.tensor.matmul(ps2[:], w2s[:, k, :], r1[:, k, :],
                                     start=(k == 0), stop=(k == RM - 1))
                lg = op.tile([E, RTOK], F32, tag="lg", name="lg")
                nc.scalar.activation(lg[:], ps2[:],
                                     mybir.ActivationFunctionType.Identity,
                                     bias=b2s[:, 0:1])
                nc.sync.dma_start(out=logits[:, :], in_=lg[:])
    nc.compile()
    _cache[key] = nc
    return nc


# -------------------------------------------------------------------- ffn ---
def _token_chunks(u):
    res = []
    off = 0
    while off < u:
        c = min(512, u - off)
        res.append((off, c))
        off += c
    return res


def _build_ffn(u, repeat=1):
    key = ("ffn", u, repeat)
    if key in _cache:
        return _cache[key]
    nc = bacc.Bacc("TRN2", target_bir_lowering=False, debug=False,
                   enable_asserts=False, num_devices=NC)
    xt16 = nc.dram_tensor("xt16", [H, u], BF16, kind="ExternalInput").ap()
    xres = nc.dram_tensor("xres", [H, u], F32, kind="ExternalInput").ap()
    w1t = nc.dram_tensor("w1t", [FM, 128, HK, 128], BF16, kind="ExternalInput").ap()
    w2t = nc.dram_tensor("w2t", [HK, 128, FM, 128], BF16, kind="ExternalInput").ap()
    b1t = nc.dram_tensor("b1t", [128, FM], F32, kind="ExternalInput").ap()
    b2t = nc.dram_tensor("b2t", [128, HK], F32, kind="ExternalInput").ap()
    yt = nc.dram_tensor("yt", [H, u], F32, kind="ExternalOutput").ap()

    tch = _token_chunks(u)

    with tile.TileContext(nc) as tc:
        with tc.tile_pool(name="xp", bufs=1) as xp, \
             tc.tile_pool(name="cp", bufs=1) as cp, \
             tc.tile_pool(name="w1p", bufs=4) as w1p, \
             tc.tile_pool(name="w2p", bufs=3) as w2p, \
             tc.tile_pool(name="midp", bufs=1) as midp, \
             tc.tile_pool(name="xrp", bufs=2) as xrp, \
             tc.tile_pool(name="yp", bufs=2) as yp, \
             tc.tile_pool(name="ps1", bufs=2, space="PSUM") as ps1p, \
             tc.tile_pool(name="ps2", bufs=2, space="PSUM") as ps2p:
            xtile = xp.tile([128, HK, u], BF16)
            nc.sync.dma_start(out=xtile[:],
                              in_=xt16.rearrange("(kt p) n -> p kt n", p=128))
            b1s = cp.tile([128, FM], F32)
            nc.sync.dma_start(out=b1s[:], in_=b1t[:, :])
            b2s = cp.tile([128, HK], F32)
            nc.sync.dma_start(out=b2s[:], in_=b2t[:, :])

            mid = midp.tile([128, FM, u], BF16)

            for _rep in range(repeat):
                _ffn_body(nc, tc, u, tch, xtile, b1s, b2s, mid,
                          w1p, w2p, xrp, yp, ps1p, ps2p, w1t, w2t, xres, yt)
    nc.compile()
    _cache[key] = nc
    return nc


def _ffn_body(nc, tc, u, tch, xtile, b1s, b2s, mid,
              w1p, w2p, xrp, yp, ps1p, ps2p, w1t, w2t, xres, yt):
            # --- mm1: mid^T = gelu(W1^T @ X^T + b1), F on partitions ---
            for m in range(FM):
                wslab = w1p.tile([128, HK, 128], BF16)
                nc.sync.dma_start(out=wslab[:], in_=w1t[m])
                pss = [ps1p.tile([128, c], F32, tag=f"ps1_{ci}", name=f"ps1t_{ci}")
                       for ci, (o, c) in enumerate(tch)]
                for k in range(HK):
                    for ci, (o, c) in enumerate(tch):
                        nc.tensor.matmul(pss[ci][:], wslab[:, k, :],
                                         xtile[:, k, o:o + c],
                                         start=(k == 0), stop=(k == HK - 1))
                for ci, (o, c) in enumerate(tch):
                    nc.scalar.activation(mid[:, m, o:o + c], pss[ci][:],
                                         mybir.ActivationFunctionType.Gelu,
                                         bias=b1s[:, m:m+1])

            # --- mm2: y^T = W2^T @ mid^T + b2 + x^T, H on partitions ---
            for m2 in range(HK):
                w2slab = w2p.tile([128, FM, 128], BF16)
                nc.sync.dma_start(out=w2slab[:], in_=w2t[m2])
                pss = [ps2p.tile([128, c], F32, tag=f"ps2_{ci}", name=f"ps2t_{ci}")
                       for ci, (o, c) in enumerate(tch)]
                for k2 in range(FM):
                    for ci, (o, c) in enumerate(tch):
                        nc.tensor.matmul(pss[ci][:], w2slab[:, k2, :],
                                         mid[:, k2, o:o + c],
                                         start=(k2 == 0), stop=(k2 == FM - 1))
                xr = xrp.tile([128, u], F32)
                nc.sync.dma_start(out=xr[:], in_=xres[m2 * 128:(m2 + 1) * 128, :])
                ytile = yp.tile([128, u], F32)
                for ci, (o, c) in enumerate(tch):
                    nc.scalar.activation(ytile[:, o:o + c], pss[ci][:],
                                         mybir.ActivationFunctionType.Identity,
                                         bias=b2s[:, m2:m2+1])
                nc.vector.tensor_add(ytile[:], ytile[:], xr[:])
                nc.sync.dma_start(out=yt[m2 * 128:(m2 + 1) * 128, :], in_=ytile[:])


# ------------------------------------------------------------------- plan ---
def _plan(counts):
    """Pick uniform per-core token quota u and expert-to-core assignment."""
    u = ((N // NC + 15) // 16) * 16
    while True:
        g = [(c + u - 1) // u if c > 0 else 0 for c in counts]
        if sum(g) <= NC:
            break
        u += 16
    assignments = []            # list of (expert, slice_index_within_expert)
    for e in range(E):
        for i in range((counts[e] + u - 1) // u if counts[e] > 0 else 0):
            assignments.append((e, i))
    # spare cores: duplicate work of the largest expert (results discarded)
    emax = int(np.argmax(counts))
    while len(assignments) < NC:
        assignments.append((emax, 0))
    return u, assignments[:NC]


# ----------------------------------------------------------------- kernel ---
def kernel(hidden_states, wr1, br1, wr2, br2, w1, b1, w2, b2):
    x = np.ascontiguousarray(np.asarray(hidden_states, dtype=np.float32).reshape(N, H))
    xT = np.ascontiguousarray(x.T)                      # [H, N] fp32

    # ---------- Launch A: router ----------
    ncA = _build_router()
    wr1 = np.asarray(wr1, dtype=np.float32)
    wr1t = np.ascontiguousarray(
        wr1.reshape(HK, 128, RM, 128).transpose(2, 1, 0, 3))
    br1t = np.ascontiguousarray(np.asarray(br1, np.float32).reshape(RM, 128).T)
    wr2t = np.ascontiguousarray(
        np.asarray(wr2, np.float32).reshape(RM, 128, E).transpose(1, 0, 2))
    br2t = np.asarray(br2, np.float32).reshape(E, 1)
    in_maps = []
    for c in range(NC):
        in_maps.append({
            "xt": np.ascontiguousarray(xT[:, c * RTOK:(c + 1) * RTOK]),
            "wr1t": wr1t, "br1t": br1t, "wr2t": wr2t, "br2t": br2t,
        })
    resA = run_bass_kernel_spmd(ncA, in_maps, list(range(NC)))
    logitsT = np.concatenate([resA.results[c]["logits"] for c in range(NC)], axis=1)
    logits = np.ascontiguousarray(logitsT.T)            # [N, E] fp32

    # ---------- Host: route + plan ----------
    idx = np.argmax(logits, axis=1)
    counts = np.bincount(idx, minlength=E)
    u, assignments = _plan(counts)
    order = np.argsort(idx, kind="stable")
    starts = np.concatenate([[0], np.cumsum(counts)])
    expert_tokens = [order[starts[e]:starts[e + 1]] for e in range(E)]

    # ---------- Launch B: expert FFN ----------
    ncB = _build_ffn(u)
    w1 = np.asarray(w1, np.float32)
    w2 = np.asarray(w2, np.float32)
    b1 = np.asarray(b1, np.float32)
    b2 = np.asarray(b2, np.float32)
    w1t_e, w2t_e, b1t_e, b2t_e = {}, {}, {}, {}
    for e in set(a[0] for a in assignments):
        w1t_e[e] = np.ascontiguousarray(
            w1[e].astype(BF).reshape(HK, 128, FM, 128).transpose(2, 1, 0, 3))
        w2t_e[e] = np.ascontiguousarray(
            w2[e].astype(BF).reshape(FM, 128, HK, 128).transpose(2, 1, 0, 3))
        b1t_e[e] = np.ascontiguousarray(b1[e].reshape(FM, 128).T)
        b2t_e[e] = np.ascontiguousarray(b2[e].reshape(HK, 128).T)

    core_tokens = []     # token ids per core (len u, padded with repeats)
    core_valid = []      # number of real tokens for this core
    in_maps = []
    for c, (e, i) in enumerate(assignments):
        toks = expert_tokens[e][i * u:(i + 1) * u]
        nvalid = len(toks)
        if nvalid < u:
            pad_src = expert_tokens[e][:1] if nvalid == 0 else toks[-1:]
            toks = np.concatenate([toks, np.repeat(pad_src, u - nvalid)])
        core_tokens.append(toks)
        core_valid.append(nvalid)
        xs = xT[:, toks]                                # [H, u] fp32
        in_maps.append({
            "xt16": np.ascontiguousarray(xs.astype(BF)),
            "xres": np.ascontiguousarray(xs),
            "w1t": w1t_e[e], "w2t": w2t_e[e],
            "b1t": b1t_e[e], "b2t": b2t_e[e],
        })
    resB = run_bass_kernel_spmd(ncB, in_maps, list(range(NC)))

    # ---------- Host: unshard ----------
    out = np.empty((N, H), dtype=np.float32)
    for c in range(NC):
        nvalid = core_valid[c]
        if nvalid == 0:
            continue
        ytc = resB.results[c]["yt"]                     # [H, u] fp32
        out[core_tokens[c][:nvalid]] = ytc[:, :nvalid].T
    return out.reshape(B, S, H), logits.reshape(B, S, E)
